# revision 7
# baseline (speedup 1.0000x reference)
"""Trainium2 Bass kernel for a custom transformer block (v2, bf16).

Sharding: 8 cores = 4 batches x 2 sequence halves. Each core computes the
full block (LN1 -> QKV -> windowed attention -> LN2 -> MLP -> residual) for
its 1024 query tokens; the KV window (last 1024 tokens of its batch) is
recomputed on both cores of a batch pair to avoid any collectives.

v2 changes vs the fp32r baseline:
- all matmuls in bf16 (weights host-cast; LN gain/bias folded into the
  weight matrices and effective biases on the host, so device LN is just
  (x-mu)*rstd).
- all 128x128 transposes moved off the PE onto the DMA xbar
  (dma_start_transpose), with contiguous destinations.
- padding mask applied as a per-key bias inside the exp activation
  (exp(s-200)=0 exactly); the causal mask is a bf16 min against
  exp-domain constants applied only on the staircase strip tiles.
- attention probs, q/k/v, z in bf16; psum evacuations on DVE.
- MLP h2 accumulated in PSUM across all 32 contraction chunks (w2 is
  streamed twice, once per query half) with the residual fused into the
  final evacuation.
"""
import sys
import os

if "/opt/trn_rl_repo" not in sys.path:
    sys.path.insert(0, "/opt/trn_rl_repo")

import numpy as np
import ml_dtypes

B, S, D = 4, 2048, 1024
N_HEAD = 16
D_HEAD = 64
WINDOW = 1024
D_FF = 4096
EPS = 1e-5
ISD = float(1.0 / np.sqrt(D))  # 1/32
EMASK = float(np.exp(-80.0))   # exp-domain mask value for causal-masked
EKEEP = 3e38
PADB = -200.0                  # pad-key bias inside exp: exp(s-200) == 0.0
P = 128

_CACHE = {}


def _build_program():
    import concourse.bacc as bacc
    import concourse.mybir as mybir
    from concourse.tile import TileContext

    F32 = mybir.dt.float32
    BF16 = mybir.dt.bfloat16
    AF = mybir.ActivationFunctionType
    ALU = mybir.AluOpType
    AX = mybir.AxisListType

    nc = bacc.Bacc("TRN2", target_bir_lowering=False, debug=False,
                   num_devices=8)

    xin_d = nc.dram_tensor("xin", [2 * WINDOW, D], F32, kind="ExternalInput")
    maskT_d = nc.dram_tensor("maskT", [WINDOW, WINDOW], BF16,
                             kind="ExternalInput")
    wq_d = nc.dram_tensor("wq", [D, D], BF16, kind="ExternalInput")
    wkv_d = nc.dram_tensor("wkv", [D, 2 * D], BF16, kind="ExternalInput")
    w1_d = nc.dram_tensor("w1", [D, D_FF], BF16, kind="ExternalInput")
    w2_d = nc.dram_tensor("w2", [D_FF, D], BF16, kind="ExternalInput")
    bqs_d = nc.dram_tensor("bqs", [P, 8], F32, kind="ExternalInput")
    bkvk_d = nc.dram_tensor("bkvk", [P, 8], F32, kind="ExternalInput")
    bkvvb_d = nc.dram_tensor("bkvvb", [P, D], F32, kind="ExternalInput")
    colb_d = nc.dram_tensor("colb", [P, 8], F32, kind="ExternalInput")
    b1s_d = nc.dram_tensor("b1s", [P, 32], F32, kind="ExternalInput")
    b2s_d = nc.dram_tensor("b2s", [P, 8], F32, kind="ExternalInput")
    xinT_d = nc.dram_tensor("xinT", [D, WINDOW], F32, kind="ExternalInput")
    y_d = nc.dram_tensor("y", [D, WINDOW], F32, kind="ExternalOutput")

    with TileContext(nc) as tc:
        cpool = tc.alloc_tile_pool(name="const", bufs=1, side="left")
        smallc = cpool.tile([P, 64], F32)
        bqs = smallc[:, 0:8]
        bkvk = smallc[:, 8:16]
        b1s = smallc[:, 16:48]
        b2s = smallc[:, 48:56]
        colb = smallc[:, 56:64]
        onesc = cpool.tile([P, 16], F32)
        nc.vector.memset(onesc[:], 1.0)
        nc.sync.dma_start(bqs, bqs_d[:])
        nc.sync.dma_start(bkvk, bkvk_d[:])
        nc.sync.dma_start(b1s, b1s_d[:])
        nc.sync.dma_start(b2s, b2s_d[:])
        nc.sync.dma_start(colb, colb_d[:])
        bkvvb = cpool.tile([P, D], F32)
        nc.sync.dma_start(bkvvb[:], bkvvb_d[:])

        # weights for QKV: issue DMA up front so they land during LN1
        wpool = tc.alloc_tile_pool(name="wqkv", bufs=1, side="left")
        wqr = wpool.tile([P, 8, D], BF16)
        wkvr = wpool.tile([P, 8, 2 * D], BF16)
        for kc in range(8):
            nc.sync.dma_start(wqr[:, kc, :], wq_d[kc * P:(kc + 1) * P, :])
            nc.sync.dma_start(wkvr[:, kc, :], wkv_d[kc * P:(kc + 1) * P, :])

        # ---------------- Phase B: LN1 + transpose to dim-major ------------
        zTp = tc.alloc_tile_pool(name="zT", bufs=1, side="left")
        zqT = zTp.tile([P, 8, WINDOW], BF16)
        zwT = zTp.tile([P, 8, WINDOW], BF16)
        xz = tc.alloc_tile_pool(name="xz", bufs=3, side="left")

        def ln1_tile(t):
            xt = xz.tile([P, D], F32, tag="x")
            nc.sync.dma_start(xt[:], xin_d[t * P:(t + 1) * P, :])
            st = xz.tile([P, 8], F32, tag="stats")
            musum, mu, sqsum = st[:, 0:1], st[:, 1:2], st[:, 2:3]
            musq, veps, sdv, rstd = st[:, 3:4], st[:, 4:5], st[:, 5:6], st[:, 6:7]
            nc.vector.reduce_sum(musum, xt[:], axis=AX.X)
            nc.vector.tensor_scalar_mul(mu, musum, 1.0 / D)
            sq = xz.tile([P, D], F32, tag="sq")
            nc.scalar.activation(sq[:], xt[:], AF.Square, accum_out=sqsum)
            nc.vector.tensor_scalar(veps, sqsum, 1.0 / D, EPS,
                                    op0=ALU.mult, op1=ALU.add)
            nc.vector.tensor_tensor(musq, mu, mu, op=ALU.mult)
            nc.vector.tensor_tensor(veps, veps, musq, op=ALU.subtract)
            nc.scalar.sqrt(sdv, veps)
            nc.vector.reciprocal(rstd, sdv)
            z = xz.tile([P, D], BF16, tag="z")
            nc.vector.tensor_scalar(z[:], xt[:], mu, rstd,
                                    op0=ALU.subtract, op1=ALU.mult)
            dst = zqT if t < 8 else zwT
            col = (t % 8) * P
            for c in range(8):
                nc.sync.dma_start_transpose(dst[:, c, col:col + P],
                                            z[:, c * P:(c + 1) * P])

        for t in range(8):
            ln1_tile(t)

        qkvp = tc.alloc_tile_pool(name="qkv", bufs=1, side="right")
        qT = qkvp.tile([P, 8, WINDOW], BF16)      # q/sqrt(D), dim-major
        kT = qkvp.tile([P, 8, WINDOW], BF16)      # k, dim-major
        V = qkvp.tile([P, 8, N_HEAD * 65], BF16)  # token-major + ones col

        psC = tc.alloc_tile_pool(name="psC", bufs=8, space="PSUM")

        # Q: weights stationary -> qT dim-major, scaled by 1/32.
        # kc outer so each loaded weight tile serves both query halves.
        for cog in range(2):
            pq = [[psC.tile([P, 512], F32, tag="proj", name=f"pq{cog}_{a}_{b}")
                   for b in range(2)] for a in range(4)]
            for kc in range(8):
                for c4 in range(4):
                    co = cog * 4 + c4
                    for qh in range(2):
                        nc.tensor.matmul(
                            pq[c4][qh][:], wqr[:, kc, co * P:(co + 1) * P],
                            zqT[:, kc, qh * 512:(qh + 1) * 512],
                            start=(kc == 0), stop=(kc == 7))
            for c4 in range(4):
                co = cog * 4 + c4
                for qh in range(2):
                    nc.vector.tensor_scalar(
                        qT[:, co, qh * 512:(qh + 1) * 512], pq[c4][qh][:],
                        ISD, bqs[:, co:co + 1], op0=ALU.mult, op1=ALU.add)

        for t in range(8, 16):
            ln1_tile(t)

        # K: weights stationary -> kT dim-major
        for cog in range(2):
            pk = [[psC.tile([P, 512], F32, tag="proj", name=f"pk{cog}_{a}_{b}")
                   for b in range(2)] for a in range(4)]
            for kc in range(8):
                for c4 in range(4):
                    co = cog * 4 + c4
                    for qh in range(2):
                        nc.tensor.matmul(
                            pk[c4][qh][:], wkvr[:, kc, co * P:(co + 1) * P],
                            zwT[:, kc, qh * 512:(qh + 1) * 512],
                            start=(kc == 0), stop=(kc == 7))
            for c4 in range(4):
                co = cog * 4 + c4
                for qh in range(2):
                    nc.vector.tensor_scalar(
                        kT[:, co, qh * 512:(qh + 1) * 512], pk[c4][qh][:],
                        bkvk[:, co:co + 1], None, op0=ALU.add)

        # V: activations stationary -> token-major, bias added at evac
        for ttg in range(2):
            pv = [[psC.tile([P, 512], F32, tag="proj", name=f"pv{ttg}_{a}_{b}")
                   for b in range(2)] for a in range(4)]
            for kc in range(8):
                for t4 in range(4):
                    tt = ttg * 4 + t4
                    for vh in range(2):
                        nc.tensor.matmul(
                            pv[t4][vh][:], zwT[:, kc, tt * P:(tt + 1) * P],
                            wkvr[:, kc, D + vh * 512:D + (vh + 1) * 512],
                            start=(kc == 0), stop=(kc == 7))
            for t4 in range(4):
                tt = ttg * 4 + t4
                for vh in range(2):
                    vdst = V[:, tt, :].rearrange("p (h n) -> p h n", n=65)[
                        :, vh * 8:(vh + 1) * 8, 0:64]
                    nc.vector.scalar_tensor_tensor(
                        vdst, pv[t4][vh][:].rearrange("p (h n) -> p h n", n=64),
                        0.0,
                        bkvvb[:, vh * 512:(vh + 1) * 512].rearrange(
                            "p (h n) -> p h n", n=64),
                        op0=ALU.add, op1=ALU.add)
        for tt in range(8):
            nc.scalar.copy(
                V[:, tt, :].rearrange("p (h n) -> p h n", n=65)[:, :, 64:65],
                onesc.rearrange("p (h n) -> p h n", n=1))

        psC.release()
        xz.release()
        zTp.release()
        wpool.release()

        # prefetch first MLP weight chunks during attention
        wf1 = tc.alloc_tile_pool(name="wf1", bufs=2, side="left")
        w1rs = {}
        for sc in range(2):
            w1r = wf1.tile([P, 8, D], BF16, tag="w1r")
            for kc in range(8):
                nc.sync.dma_start(
                    w1r[:, kc, :], w1_d[kc * P:(kc + 1) * P,
                                        sc * 1024:(sc + 1) * 1024])
            w1rs[sc] = w1r

        # ---------------- Phase D: attention --------------------------------
        attnp = tc.alloc_tile_pool(name="attn", bufs=1, side="left")
        attn = attnp.tile([P, 8, D], BF16)         # normalized attn out
        asum = attnp.tile([P, 8, N_HEAD], F32)     # per-head row sums of attn
        rinv = attnp.tile([P, 2, 8], F32)          # per-head 1/rowsum
        oa = attnp.tile([80, 2, WINDOW], BF16)     # AV out + rowsum row
        nc.vector.memset(oa[:, :, :], 0.0)

        mkp = tc.alloc_tile_pool(name="mk", bufs=1, side="left")
        maskT = mkp.tile([P, 8, WINDOW], BF16)
        nc.sync.dma_start(maskT[:], maskT_d.rearrange("(c p) n -> p c n", p=P))
        ptp = tc.alloc_tile_pool(name="ptp", bufs=2, side="left")
        otp = tc.alloc_tile_pool(name="otp", bufs=2, side="left")
        psS = tc.alloc_tile_pool(name="psS", bufs=2, space="PSUM")
        psA = tc.alloc_tile_pool(name="psA", bufs=2, space="PSUM")

        for hp in range(N_HEAD // 2):
            pair = (2 * hp, 2 * hp + 1)
            pts = {0: ptp.tile([P, 8, WINDOW], BF16, tag="pts0", name=f"pts0_{hp}"),
                   1: ptp.tile([P, 8, WINDOW], BF16, tag="pts1", name=f"pts1_{hp}")}
            avp = {0: psA.tile([65, WINDOW], F32, tag="av", name=f"av0_{hp}"),
                   1: psA.tile([65, WINDOW], F32, tag="av", name=f"av1_{hp}")}
            for kc in range(8):
                sps = {0: psS.tile([P, WINDOW], F32, tag="s", name=f"s0_{hp}_{kc}"),
                       1: psS.tile([P, WINDOW], F32, tag="s", name=f"s1_{hp}_{kc}")}
                # alternate PE row halves so LDWEIGHTS overlaps matmul
                for qh in range(2):
                    for i in range(2):
                        po = 64 * i
                        nc.tensor.matmul(
                            sps[i][:, qh * 512:(qh + 1) * 512],
                            kT[po:po + 64, hp, kc * P:(kc + 1) * P],
                            qT[po:po + 64, hp, qh * 512:(qh + 1) * 512],
                            start=True, stop=True)
                for i in range(2):
                    nc.scalar.activation(pts[i][:, kc, :], sps[i][:], AF.Exp,
                                         bias=colb[:, kc:kc + 1])
                w = min(P * (kc + 1), WINDOW)
                for i in range(2):
                    nc.vector.tensor_tensor(
                        pts[i][:, kc, 0:w], pts[i][:, kc, 0:w],
                        maskT[:, kc, 0:w], op=ALU.min)
                for i in range(2):
                    h = pair[i]
                    for qh in range(2):
                        nc.tensor.matmul(
                            avp[i][:, qh * 512:(qh + 1) * 512],
                            V[:, kc, h * 65:(h + 1) * 65],
                            pts[i][:, kc, qh * 512:(qh + 1) * 512],
                            start=(kc == 0), stop=(kc == 7))
            for i in range(2):
                h = pair[i]
                for qh in range(2):
                    nc.vector.tensor_copy(
                        oa[0:65, i, qh * 512:(qh + 1) * 512],
                        avp[i][:, qh * 512:(qh + 1) * 512])
                oT = otp.tile([P, 8, 80], BF16, tag="oT")
                for c in range(8):
                    nc.sync.dma_start_transpose(
                        oT[:, c, :], oa[:, i, c * P:(c + 1) * P])
                nc.vector.reciprocal(rinv[:, i, :], oT[:, :, 64:65])
                for t in range(8):
                    nc.vector.tensor_scalar(
                        attn[:, t, h * 64:(h + 1) * 64], oT[:, t, 0:64],
                        rinv[:, i, t:t + 1], 0.0, op0=ALU.mult,
                        op1=ALU.add, accum_out=asum[:, t, h:h + 1])

        psA.release()
        psS.release()
        otp.release()
        ptp.release()
        mkp.release()
        qkvp.release()

        # ---------------- Phase E: LN2 + transpose ---------------------------
        z2Tp = tc.alloc_tile_pool(name="z2T", bufs=1, side="right")
        z2T = z2Tp.tile([P, 8, WINDOW], BF16)
        xz2 = tc.alloc_tile_pool(name="xz2", bufs=2, side="left")
        for t in range(8):
            at = attn[:, t, :]
            st = xz2.tile([P, 8], F32, tag="stats2")
            musum, mu, vsum = st[:, 0:1], st[:, 1:2], st[:, 2:3]
            veps, sdv, rstd = st[:, 4:5], st[:, 5:6], st[:, 6:7]
            nc.vector.reduce_sum(musum, asum[:, t, :], axis=AX.X)
            nc.vector.tensor_scalar_mul(mu, musum, 1.0 / D)
            scr = xz2.tile([P, D], BF16, tag="scrE")
            nc.vector.scalar_tensor_tensor(
                scr[:], at, mu, at,
                op0=ALU.subtract, op1=ALU.mult, accum_out=vsum)
            nc.vector.tensor_scalar(veps, vsum, 1.0 / D, EPS,
                                    op0=ALU.mult, op1=ALU.add)
            nc.scalar.sqrt(sdv, veps)
            nc.vector.reciprocal(rstd, sdv)
            z2 = xz2.tile([P, D], BF16, tag="z2E")
            nc.vector.tensor_scalar(z2[:], at, mu, rstd,
                                    op0=ALU.subtract, op1=ALU.mult)
            for c in range(8):
                nc.sync.dma_start_transpose(z2T[:, c, t * P:(t + 1) * P],
                                            z2[:, c * P:(c + 1) * P])

        xz2.release()
        attnp.release()

        # ---------------- Phase F: MLP ---------------------------------------
        h1p = tc.alloc_tile_pool(name="h1p", bufs=1, side="left")
        h1 = h1p.tile([P, 32, WINDOW], BF16)
        psF = tc.alloc_tile_pool(name="psF", bufs=8, space="PSUM")

        for sc in range(4):
            if sc in w1rs:
                w1r = w1rs.pop(sc)
            else:
                w1r = wf1.tile([P, 8, D], BF16, tag="w1r")
                for kc in range(8):
                    nc.sync.dma_start(
                        w1r[:, kc, :], w1_d[kc * P:(kc + 1) * P,
                                            sc * 1024:(sc + 1) * 1024])
            for ftg in range(2):
                ph = [[psF.tile([P, 512], F32, tag="f", name=f"ph{sc}_{ftg}_{a}_{b}")
                       for b in range(2)] for a in range(4)]
                for kc in range(8):
                    for f4 in range(4):
                        ft = ftg * 4 + f4
                        for qh in range(2):
                            nc.tensor.matmul(
                                ph[f4][qh][:],
                                w1r[:, kc, ft * P:(ft + 1) * P],
                                z2T[:, kc, qh * 512:(qh + 1) * 512],
                                start=(kc == 0), stop=(kc == 7))
                for f4 in range(4):
                    ft = sc * 8 + ftg * 4 + f4
                    for qh in range(2):
                        nc.scalar.activation(
                            h1[:, ft, qh * 512:(qh + 1) * 512], ph[f4][qh][:],
                            AF.Silu, bias=b1s[:, ft:ft + 1], scale=1.0)

        z2Tp.release()

        # h2: accumulate all 32 contraction chunks in PSUM per (co, qh);
        # w2 is streamed once per query half.
        wf2 = tc.alloc_tile_pool(name="wf2", bufs=2, side="right")
        tailp = tc.alloc_tile_pool(name="tail", bufs=3, side="left")
        for qh in range(2):
            ph2 = [psF.tile([P, 512], F32, tag="f", name=f"ph2_{qh}_{a}") for a in range(8)]
            for sc in range(4):
                w2r = wf2.tile([P, 8, D], BF16, tag="w2r")
                for kc in range(8):
                    nc.sync.dma_start(
                        w2r[:, kc, :],
                        w2_d[(sc * 8 + kc) * P:(sc * 8 + kc + 1) * P, :])
                for kc in range(8):
                    for co in range(8):
                        nc.tensor.matmul(
                            ph2[co][:], w2r[:, kc, co * P:(co + 1) * P],
                            h1[:, sc * 8 + kc, qh * 512:(qh + 1) * 512],
                            start=(sc == 0 and kc == 0),
                            stop=(sc == 3 and kc == 7))
            for co in range(8):
                xq = tailp.tile([P, 512], F32, tag="xq")
                nc.sync.dma_start(
                    xq[:], xinT_d[co * P:(co + 1) * P,
                                  qh * 512:(qh + 1) * 512])
                y = tailp.tile([P, 512], F32, tag="y")
                nc.vector.scalar_tensor_tensor(
                    y[:], ph2[co][:], b2s[:, co:co + 1], xq[:],
                    op0=ALU.add, op1=ALU.add)
                nc.sync.dma_start(
                    y_d[co * P:(co + 1) * P, qh * 512:(qh + 1) * 512], y[:])

        psF.release()
        tailp.release()
        h1p.release()
        wf1.release()
        wf2.release()
        cpool.release()

    nc.compile()
    return nc


def _prep_inputs(inputs):
    x = np.ascontiguousarray(np.asarray(inputs["x"], dtype=np.float32))
    kpm = np.asarray(inputs["key_pad_mask"]).astype(bool)
    wq = np.asarray(inputs["wq"], dtype=np.float32)
    wkv = np.asarray(inputs["wkv"], dtype=np.float32)
    w1 = np.asarray(inputs["w1"], dtype=np.float32)
    w2 = np.asarray(inputs["w2"], dtype=np.float32)
    bq = np.asarray(inputs["bq"], dtype=np.float32)
    bkv = np.asarray(inputs["bkv"], dtype=np.float32)
    b1 = np.asarray(inputs["b1"], dtype=np.float32)
    b2 = np.asarray(inputs["b2"], dtype=np.float32)
    ln1_g = np.asarray(inputs["ln1_g"], dtype=np.float32)
    ln1_b = np.asarray(inputs["ln1_b"], dtype=np.float32)
    ln2_g = np.asarray(inputs["ln2_g"], dtype=np.float32)
    ln2_b = np.asarray(inputs["ln2_b"], dtype=np.float32)

    # fold LN gains into the weights and LN biases into effective biases
    wq_f = wq * ln1_g[:, None]
    wkv_f = wkv * ln1_g[:, None]
    w1_f = w1 * ln2_g[:, None]
    bq_eff = ln1_b @ wq + bq
    bkv_eff = ln1_b @ wkv + bkv
    b1_eff = ln2_b @ w1 + b1

    def bf(a):
        return np.ascontiguousarray(a.astype(ml_dtypes.bfloat16))

    def dm(v):  # [D] -> [P, 8] dim-major chunk layout
        return np.ascontiguousarray(v.reshape(8, P).T)

    shared = {
        "wq": bf(wq_f),
        "wkv": bf(wkv_f),
        "w1": bf(w1_f),
        "w2": bf(w2),
        "bqs": np.ascontiguousarray((bq_eff * ISD).reshape(8, P).T),
        "bkvk": dm(bkv_eff[0:D]),
        "bkvvb": np.ascontiguousarray(
            np.broadcast_to(bkv_eff[D:2 * D], (P, D)).astype(np.float32)),
        "b1s": np.ascontiguousarray(b1_eff.reshape(32, P).T),
        "b2s": dm(b2),
    }

    j = np.arange(WINDOW)[:, None]   # key index within window (row)
    i = np.arange(WINDOW)[None, :]   # local query index (col)
    in_maps = []
    for core in range(8):
        b, h = core // 2, core % 2
        xq = x[b, h * WINDOW:(h + 1) * WINDOW]
        xw = x[b, S - WINDOW:S]
        causal = j > h * WINDOW + i
        maskT = np.where(causal, np.float32(EMASK),
                         np.float32(EKEEP)).astype(ml_dtypes.bfloat16)
        colb = np.where(kpm[b, S - WINDOW:S], np.float32(PADB),
                        np.float32(0.0)).reshape(8, P).T
        m = dict(shared)
        m["xin"] = np.ascontiguousarray(np.concatenate([xq, xw], axis=0))
        m["xinT"] = np.ascontiguousarray(xq.T)
        m["maskT"] = np.ascontiguousarray(maskT)
        m["colb"] = np.ascontiguousarray(colb)
        in_maps.append(m)
    return in_maps


def kernel(**inputs):
    from concourse.bass_utils import run_bass_kernel_spmd

    if "nc" not in _CACHE:
        _CACHE["nc"] = _build_program()
    nc = _CACHE["nc"]

    in_maps = _prep_inputs(inputs)
    trace = os.environ.get("KERNEL_TRACE", "0") == "1"
    res = run_bass_kernel_spmd(nc, in_maps, core_ids=list(range(8)),
                               trace=trace)
    if res.exec_time_ns is not None:
        print(f"HW exec time: {res.exec_time_ns} ns")
        _CACHE["exec_time_ns"] = res.exec_time_ns
    out = np.empty((B, S, D), dtype=np.float32)
    for core in range(8):
        b, h = core // 2, core % 2
        out[b, h * WINDOW:(h + 1) * WINDOW] = res.results[core]["y"].T
    return out


# revision 8
# speedup vs baseline: 1.6426x; 1.6426x over previous
"""Trainium2 Bass kernel for a custom transformer block (v2, bf16).

Sharding: 8 cores = 4 batches x 2 sequence halves. Each core computes the
full block (LN1 -> QKV -> windowed attention -> LN2 -> MLP -> residual) for
its 1024 query tokens; the KV window (last 1024 tokens of its batch) is
recomputed on both cores of a batch pair to avoid any collectives.

v2 changes vs the fp32r baseline:
- all matmuls in bf16 (weights host-cast; LN gain/bias folded into the
  weight matrices and effective biases on the host, so device LN is just
  (x-mu)*rstd).
- all 128x128 transposes moved off the PE onto the DMA xbar
  (dma_start_transpose), with contiguous destinations.
- padding mask applied as a per-key bias inside the exp activation
  (exp(s-200)=0 exactly); the causal mask is a bf16 min against
  exp-domain constants applied only on the staircase strip tiles.
- attention probs, q/k/v, z in bf16; psum evacuations on DVE.
- MLP h2 accumulated in PSUM across all 32 contraction chunks (w2 is
  streamed twice, once per query half) with the residual fused into the
  final evacuation.
"""
import sys
import os

if "/opt/trn_rl_repo" not in sys.path:
    sys.path.insert(0, "/opt/trn_rl_repo")

import numpy as np
import ml_dtypes

B, S, D = 4, 2048, 1024
N_HEAD = 16
D_HEAD = 64
WINDOW = 1024
D_FF = 4096
EPS = 1e-5
ISD = float(1.0 / np.sqrt(D))  # 1/32
EMASK = float(np.exp(-80.0))   # exp-domain mask value for causal-masked
EKEEP = 3e38
PADB = -200.0                  # pad-key bias inside exp: exp(s-200) == 0.0
P = 128

_CACHE = {}


def _build_program():
    import concourse.bacc as bacc
    import concourse.mybir as mybir
    from concourse.tile import TileContext

    F32 = mybir.dt.float32
    BF16 = mybir.dt.bfloat16
    AF = mybir.ActivationFunctionType
    ALU = mybir.AluOpType
    AX = mybir.AxisListType

    nc = bacc.Bacc("TRN2", target_bir_lowering=False, debug=False,
                   num_devices=8)

    xin_d = nc.dram_tensor("xin", [2 * WINDOW, D], F32, kind="ExternalInput")
    maskT_d = nc.dram_tensor("maskT", [WINDOW, WINDOW], BF16,
                             kind="ExternalInput")
    wq_d = nc.dram_tensor("wq", [D, D], BF16, kind="ExternalInput")
    wkv_d = nc.dram_tensor("wkv", [D, 2 * D], BF16, kind="ExternalInput")
    w1_d = nc.dram_tensor("w1", [D, D_FF], BF16, kind="ExternalInput")
    w2_d = nc.dram_tensor("w2", [D_FF, D], BF16, kind="ExternalInput")
    bqs_d = nc.dram_tensor("bqs", [P, 8], F32, kind="ExternalInput")
    bkvk_d = nc.dram_tensor("bkvk", [P, 8], F32, kind="ExternalInput")
    bkvvb_d = nc.dram_tensor("bkvvb", [P, D], F32, kind="ExternalInput")
    colb_d = nc.dram_tensor("colb", [P, 8], F32, kind="ExternalInput")
    b1s_d = nc.dram_tensor("b1s", [P, 32], F32, kind="ExternalInput")
    b2s_d = nc.dram_tensor("b2s", [P, 8], F32, kind="ExternalInput")
    xinT_d = nc.dram_tensor("xinT", [D, WINDOW], F32, kind="ExternalInput")
    y_d = nc.dram_tensor("y", [D, WINDOW], F32, kind="ExternalOutput")

    with TileContext(nc) as tc:
        cpool = tc.alloc_tile_pool(name="const", bufs=1, side="left")
        smallc = cpool.tile([P, 64], F32)
        bqs = smallc[:, 0:8]
        bkvk = smallc[:, 8:16]
        b1s = smallc[:, 16:48]
        b2s = smallc[:, 48:56]
        colb = smallc[:, 56:64]
        onesc = cpool.tile([P, 16], F32)
        nc.vector.memset(onesc[:], 1.0)
        nc.sync.dma_start(bqs, bqs_d[:])
        nc.sync.dma_start(bkvk, bkvk_d[:])
        nc.sync.dma_start(b1s, b1s_d[:])
        nc.sync.dma_start(b2s, b2s_d[:])
        nc.sync.dma_start(colb, colb_d[:])
        bkvvb = cpool.tile([P, D], F32)
        nc.sync.dma_start(bkvvb[:], bkvvb_d[:])

        # weights for QKV: issue DMA up front so they land during LN1
        wpool = tc.alloc_tile_pool(name="wqkv", bufs=1, side="left")
        wqr = wpool.tile([P, 8, D], BF16)
        wkvr = wpool.tile([P, 8, 2 * D], BF16)
        for kc in range(8):
            nc.sync.dma_start(wqr[:, kc, :], wq_d[kc * P:(kc + 1) * P, :])
            nc.sync.dma_start(wkvr[:, kc, :], wkv_d[kc * P:(kc + 1) * P, :])

        # ---------------- Phase B: LN1 + transpose to dim-major ------------
        zTp = tc.alloc_tile_pool(name="zT", bufs=1, side="left")
        zqT = zTp.tile([P, 8, 8, P], BF16)   # [dim, tok-tile, dim-chunk, tok]
        zwT = zTp.tile([P, 8, 8, P], BF16)
        xz = tc.alloc_tile_pool(name="xz", bufs=3, side="left")

        def ln1_tile(t):
            xt = xz.tile([P, D], F32, tag="x")
            nc.sync.dma_start(xt[:], xin_d[t * P:(t + 1) * P, :])
            st = xz.tile([P, 8], F32, tag="stats")
            musum, mu, sqsum = st[:, 0:1], st[:, 1:2], st[:, 2:3]
            musq, veps, sdv, rstd = st[:, 3:4], st[:, 4:5], st[:, 5:6], st[:, 6:7]
            nc.vector.reduce_sum(musum, xt[:], axis=AX.X)
            nc.vector.tensor_scalar_mul(mu, musum, 1.0 / D)
            sq = xz.tile([P, D], F32, tag="sq")
            nc.scalar.activation(sq[:], xt[:], AF.Square, accum_out=sqsum)
            nc.vector.tensor_scalar(veps, sqsum, 1.0 / D, EPS,
                                    op0=ALU.mult, op1=ALU.add)
            nc.vector.tensor_tensor(musq, mu, mu, op=ALU.mult)
            nc.vector.tensor_tensor(veps, veps, musq, op=ALU.subtract)
            nc.scalar.sqrt(sdv, veps)
            nc.vector.reciprocal(rstd, sdv)
            z = xz.tile([P, D], BF16, tag="z")
            nc.vector.tensor_scalar(z[:], xt[:], mu, rstd,
                                    op0=ALU.subtract, op1=ALU.mult)
            dst = zqT if t < 8 else zwT
            nc.sync.dma_start_transpose(dst[:, t % 8, :, :], z[:])

        for t in range(8):
            ln1_tile(t)

        qkvp = tc.alloc_tile_pool(name="qkv", bufs=1, side="right")
        qT = qkvp.tile([P, 8, WINDOW], BF16)      # q/sqrt(D), dim-major
        kT = qkvp.tile([P, 8, WINDOW], BF16)      # k, dim-major
        V = qkvp.tile([P, 8, N_HEAD * 65], BF16)  # token-major + ones col

        psC = tc.alloc_tile_pool(name="psC", bufs=8, space="PSUM")

        # Q: weights stationary -> qT dim-major, scaled by 1/32.
        # kc outer so each loaded weight tile serves both query halves.
        for cog in range(2):
            pq = [[psC.tile([P, 512], F32, tag="proj", name=f"pq{cog}_{a}_{b}")
                   for b in range(2)] for a in range(4)]
            for kc in range(8):
                for c4 in range(4):
                    co = cog * 4 + c4
                    for qh in range(2):
                        nc.tensor.matmul(
                            pq[c4][qh][:], wqr[:, kc, co * P:(co + 1) * P],
                            zqT[:, qh * 4:(qh + 1) * 4, kc, :],
                            start=(kc == 0), stop=(kc == 7))
            for c4 in range(4):
                co = cog * 4 + c4
                for qh in range(2):
                    nc.vector.tensor_scalar(
                        qT[:, co, qh * 512:(qh + 1) * 512], pq[c4][qh][:],
                        ISD, bqs[:, co:co + 1], op0=ALU.mult, op1=ALU.add)

        for t in range(8, 16):
            ln1_tile(t)

        # K: weights stationary -> kT dim-major
        for cog in range(2):
            pk = [[psC.tile([P, 512], F32, tag="proj", name=f"pk{cog}_{a}_{b}")
                   for b in range(2)] for a in range(4)]
            for kc in range(8):
                for c4 in range(4):
                    co = cog * 4 + c4
                    for qh in range(2):
                        nc.tensor.matmul(
                            pk[c4][qh][:], wkvr[:, kc, co * P:(co + 1) * P],
                            zwT[:, qh * 4:(qh + 1) * 4, kc, :],
                            start=(kc == 0), stop=(kc == 7))
            for c4 in range(4):
                co = cog * 4 + c4
                for qh in range(2):
                    nc.vector.tensor_scalar(
                        kT[:, co, qh * 512:(qh + 1) * 512], pk[c4][qh][:],
                        bkvk[:, co:co + 1], None, op0=ALU.add)

        # V: activations stationary -> token-major, bias added at evac
        for ttg in range(2):
            pv = [[psC.tile([P, 512], F32, tag="proj", name=f"pv{ttg}_{a}_{b}")
                   for b in range(2)] for a in range(4)]
            for kc in range(8):
                for t4 in range(4):
                    tt = ttg * 4 + t4
                    for vh in range(2):
                        nc.tensor.matmul(
                            pv[t4][vh][:], zwT[:, tt, kc, :],
                            wkvr[:, kc, D + vh * 512:D + (vh + 1) * 512],
                            start=(kc == 0), stop=(kc == 7))
            for t4 in range(4):
                tt = ttg * 4 + t4
                for vh in range(2):
                    vdst = V[:, tt, :].rearrange("p (h n) -> p h n", n=65)[
                        :, vh * 8:(vh + 1) * 8, 0:64]
                    nc.vector.scalar_tensor_tensor(
                        vdst, pv[t4][vh][:].rearrange("p (h n) -> p h n", n=64),
                        0.0,
                        bkvvb[:, vh * 512:(vh + 1) * 512].rearrange(
                            "p (h n) -> p h n", n=64),
                        op0=ALU.add, op1=ALU.add)
        for tt in range(8):
            nc.scalar.copy(
                V[:, tt, :].rearrange("p (h n) -> p h n", n=65)[:, :, 64:65],
                onesc.rearrange("p (h n) -> p h n", n=1))

        psC.release()
        xz.release()
        zTp.release()
        wpool.release()

        # prefetch first MLP weight chunks during attention
        wf1 = tc.alloc_tile_pool(name="wf1", bufs=2, side="left")
        w1rs = {}
        for sc in range(2):
            w1r = wf1.tile([P, 8, D], BF16, tag="w1r")
            for kc in range(8):
                nc.sync.dma_start(
                    w1r[:, kc, :], w1_d[kc * P:(kc + 1) * P,
                                        sc * 1024:(sc + 1) * 1024])
            w1rs[sc] = w1r

        # ---------------- Phase D: attention --------------------------------
        attnp = tc.alloc_tile_pool(name="attn", bufs=1, side="left")
        attn = attnp.tile([P, 8, D], BF16)         # normalized attn out
        asum = attnp.tile([P, 8, N_HEAD], F32)     # per-head row sums of attn
        rinv = attnp.tile([P, 2, 8], F32)          # per-head 1/rowsum
        oa = attnp.tile([80, 2, WINDOW], BF16)     # AV out + rowsum row
        nc.vector.memset(oa[:, :, :], 0.0)

        mkp = tc.alloc_tile_pool(name="mk", bufs=1, side="left")
        maskT = mkp.tile([P, 8, WINDOW], BF16)
        nc.sync.dma_start(maskT[:], maskT_d.rearrange("(c p) n -> p c n", p=P))
        ptp = tc.alloc_tile_pool(name="ptp", bufs=2, side="left")
        otp = tc.alloc_tile_pool(name="otp", bufs=2, side="left")
        psS = tc.alloc_tile_pool(name="psS", bufs=2, space="PSUM")
        psA = tc.alloc_tile_pool(name="psA", bufs=2, space="PSUM")

        for hp in range(N_HEAD // 2):
            pair = (2 * hp, 2 * hp + 1)
            pts = {0: ptp.tile([P, 8, WINDOW], BF16, tag="pts0", name=f"pts0_{hp}"),
                   1: ptp.tile([P, 8, WINDOW], BF16, tag="pts1", name=f"pts1_{hp}")}
            avp = {0: psA.tile([65, WINDOW], F32, tag="av", name=f"av0_{hp}"),
                   1: psA.tile([65, WINDOW], F32, tag="av", name=f"av1_{hp}")}
            for kc in range(8):
                sps = {0: psS.tile([P, WINDOW], F32, tag="s", name=f"s0_{hp}_{kc}"),
                       1: psS.tile([P, WINDOW], F32, tag="s", name=f"s1_{hp}_{kc}")}
                # alternate PE row halves so LDWEIGHTS overlaps matmul
                for qh in range(2):
                    for i in range(2):
                        po = 64 * i
                        nc.tensor.matmul(
                            sps[i][:, qh * 512:(qh + 1) * 512],
                            kT[po:po + 64, hp, kc * P:(kc + 1) * P],
                            qT[po:po + 64, hp, qh * 512:(qh + 1) * 512],
                            start=True, stop=True)
                for i in range(2):
                    nc.scalar.activation(pts[i][:, kc, :], sps[i][:], AF.Exp,
                                         bias=colb[:, kc:kc + 1])
                w = min(P * (kc + 1), WINDOW)
                for i in range(2):
                    nc.vector.tensor_tensor(
                        pts[i][:, kc, 0:w], pts[i][:, kc, 0:w],
                        maskT[:, kc, 0:w], op=ALU.min)
                for i in range(2):
                    h = pair[i]
                    for qh in range(2):
                        nc.tensor.matmul(
                            avp[i][:, qh * 512:(qh + 1) * 512],
                            V[:, kc, h * 65:(h + 1) * 65],
                            pts[i][:, kc, qh * 512:(qh + 1) * 512],
                            start=(kc == 0), stop=(kc == 7))
            for i in range(2):
                h = pair[i]
                for qh in range(2):
                    nc.vector.tensor_copy(
                        oa[0:65, i, qh * 512:(qh + 1) * 512],
                        avp[i][:, qh * 512:(qh + 1) * 512])
                oT = otp.tile([P, 8, 80], BF16, tag="oT")
                nc.sync.dma_start_transpose(oT[:, :, :], oa[:, i, :])
                nc.vector.reciprocal(rinv[:, i, :], oT[:, :, 64:65])
                for t in range(8):
                    nc.vector.tensor_scalar(
                        attn[:, t, h * 64:(h + 1) * 64], oT[:, t, 0:64],
                        rinv[:, i, t:t + 1], 0.0, op0=ALU.mult,
                        op1=ALU.add, accum_out=asum[:, t, h:h + 1])

        psA.release()
        psS.release()
        otp.release()
        ptp.release()
        mkp.release()
        qkvp.release()

        # ---------------- Phase E: LN2 + transpose ---------------------------
        z2Tp = tc.alloc_tile_pool(name="z2T", bufs=1, side="right")
        z2T = z2Tp.tile([P, 8, 8, P], BF16)
        xz2 = tc.alloc_tile_pool(name="xz2", bufs=2, side="left")
        for t in range(8):
            at = attn[:, t, :]
            st = xz2.tile([P, 8], F32, tag="stats2")
            musum, mu, vsum = st[:, 0:1], st[:, 1:2], st[:, 2:3]
            veps, sdv, rstd = st[:, 4:5], st[:, 5:6], st[:, 6:7]
            nc.vector.reduce_sum(musum, asum[:, t, :], axis=AX.X)
            nc.vector.tensor_scalar_mul(mu, musum, 1.0 / D)
            scr = xz2.tile([P, D], BF16, tag="scrE")
            nc.vector.scalar_tensor_tensor(
                scr[:], at, mu, at,
                op0=ALU.subtract, op1=ALU.mult, accum_out=vsum)
            nc.vector.tensor_scalar(veps, vsum, 1.0 / D, EPS,
                                    op0=ALU.mult, op1=ALU.add)
            nc.scalar.sqrt(sdv, veps)
            nc.vector.reciprocal(rstd, sdv)
            z2 = xz2.tile([P, D], BF16, tag="z2E")
            nc.vector.tensor_scalar(z2[:], at, mu, rstd,
                                    op0=ALU.subtract, op1=ALU.mult)
            nc.sync.dma_start_transpose(z2T[:, t, :, :], z2[:])

        xz2.release()
        attnp.release()

        # ---------------- Phase F: MLP ---------------------------------------
        h1p = tc.alloc_tile_pool(name="h1p", bufs=1, side="left")
        h1 = h1p.tile([P, 32, WINDOW], BF16)
        psF = tc.alloc_tile_pool(name="psF", bufs=8, space="PSUM")

        for sc in range(4):
            if sc in w1rs:
                w1r = w1rs.pop(sc)
            else:
                w1r = wf1.tile([P, 8, D], BF16, tag="w1r")
                for kc in range(8):
                    nc.sync.dma_start(
                        w1r[:, kc, :], w1_d[kc * P:(kc + 1) * P,
                                            sc * 1024:(sc + 1) * 1024])
            for ftg in range(2):
                ph = [[psF.tile([P, 512], F32, tag="f", name=f"ph{sc}_{ftg}_{a}_{b}")
                       for b in range(2)] for a in range(4)]
                for kc in range(8):
                    for f4 in range(4):
                        ft = ftg * 4 + f4
                        for qh in range(2):
                            nc.tensor.matmul(
                                ph[f4][qh][:],
                                w1r[:, kc, ft * P:(ft + 1) * P],
                                z2T[:, qh * 4:(qh + 1) * 4, kc, :],
                                start=(kc == 0), stop=(kc == 7))
                for f4 in range(4):
                    ft = sc * 8 + ftg * 4 + f4
                    for qh in range(2):
                        nc.scalar.activation(
                            h1[:, ft, qh * 512:(qh + 1) * 512], ph[f4][qh][:],
                            AF.Silu, bias=b1s[:, ft:ft + 1], scale=1.0)

        z2Tp.release()

        # h2: accumulate all 32 contraction chunks in PSUM per (co, qh);
        # w2 is streamed once per query half.
        wf2 = tc.alloc_tile_pool(name="wf2", bufs=2, side="right")
        tailp = tc.alloc_tile_pool(name="tail", bufs=3, side="left")
        for qh in range(2):
            ph2 = [psF.tile([P, 512], F32, tag="f", name=f"ph2_{qh}_{a}") for a in range(8)]
            for sc in range(4):
                w2r = wf2.tile([P, 8, D], BF16, tag="w2r")
                for kc in range(8):
                    nc.sync.dma_start(
                        w2r[:, kc, :],
                        w2_d[(sc * 8 + kc) * P:(sc * 8 + kc + 1) * P, :])
                for kc in range(8):
                    for co in range(8):
                        nc.tensor.matmul(
                            ph2[co][:], w2r[:, kc, co * P:(co + 1) * P],
                            h1[:, sc * 8 + kc, qh * 512:(qh + 1) * 512],
                            start=(sc == 0 and kc == 0),
                            stop=(sc == 3 and kc == 7))
            for co in range(8):
                xq = tailp.tile([P, 512], F32, tag="xq")
                nc.sync.dma_start(
                    xq[:], xinT_d[co * P:(co + 1) * P,
                                  qh * 512:(qh + 1) * 512])
                y = tailp.tile([P, 512], F32, tag="y")
                nc.vector.scalar_tensor_tensor(
                    y[:], ph2[co][:], b2s[:, co:co + 1], xq[:],
                    op0=ALU.add, op1=ALU.add)
                nc.sync.dma_start(
                    y_d[co * P:(co + 1) * P, qh * 512:(qh + 1) * 512], y[:])

        psF.release()
        tailp.release()
        h1p.release()
        wf1.release()
        wf2.release()
        cpool.release()

    nc.compile()
    return nc


def _prep_inputs(inputs):
    x = np.ascontiguousarray(np.asarray(inputs["x"], dtype=np.float32))
    kpm = np.asarray(inputs["key_pad_mask"]).astype(bool)
    wq = np.asarray(inputs["wq"], dtype=np.float32)
    wkv = np.asarray(inputs["wkv"], dtype=np.float32)
    w1 = np.asarray(inputs["w1"], dtype=np.float32)
    w2 = np.asarray(inputs["w2"], dtype=np.float32)
    bq = np.asarray(inputs["bq"], dtype=np.float32)
    bkv = np.asarray(inputs["bkv"], dtype=np.float32)
    b1 = np.asarray(inputs["b1"], dtype=np.float32)
    b2 = np.asarray(inputs["b2"], dtype=np.float32)
    ln1_g = np.asarray(inputs["ln1_g"], dtype=np.float32)
    ln1_b = np.asarray(inputs["ln1_b"], dtype=np.float32)
    ln2_g = np.asarray(inputs["ln2_g"], dtype=np.float32)
    ln2_b = np.asarray(inputs["ln2_b"], dtype=np.float32)

    # fold LN gains into the weights and LN biases into effective biases
    wq_f = wq * ln1_g[:, None]
    wkv_f = wkv * ln1_g[:, None]
    w1_f = w1 * ln2_g[:, None]
    bq_eff = ln1_b @ wq + bq
    bkv_eff = ln1_b @ wkv + bkv
    b1_eff = ln2_b @ w1 + b1

    def bf(a):
        return np.ascontiguousarray(a.astype(ml_dtypes.bfloat16))

    def dm(v):  # [D] -> [P, 8] dim-major chunk layout
        return np.ascontiguousarray(v.reshape(8, P).T)

    shared = {
        "wq": bf(wq_f),
        "wkv": bf(wkv_f),
        "w1": bf(w1_f),
        "w2": bf(w2),
        "bqs": np.ascontiguousarray((bq_eff * ISD).reshape(8, P).T),
        "bkvk": dm(bkv_eff[0:D]),
        "bkvvb": np.ascontiguousarray(
            np.broadcast_to(bkv_eff[D:2 * D], (P, D)).astype(np.float32)),
        "b1s": np.ascontiguousarray(b1_eff.reshape(32, P).T),
        "b2s": dm(b2),
    }

    j = np.arange(WINDOW)[:, None]   # key index within window (row)
    i = np.arange(WINDOW)[None, :]   # local query index (col)
    in_maps = []
    for core in range(8):
        b, h = core // 2, core % 2
        xq = x[b, h * WINDOW:(h + 1) * WINDOW]
        xw = x[b, S - WINDOW:S]
        causal = j > h * WINDOW + i
        maskT = np.where(causal, np.float32(EMASK),
                         np.float32(EKEEP)).astype(ml_dtypes.bfloat16)
        colb = np.where(kpm[b, S - WINDOW:S], np.float32(PADB),
                        np.float32(0.0)).reshape(8, P).T
        m = dict(shared)
        m["xin"] = np.ascontiguousarray(np.concatenate([xq, xw], axis=0))
        m["xinT"] = np.ascontiguousarray(xq.T)
        m["maskT"] = np.ascontiguousarray(maskT)
        m["colb"] = np.ascontiguousarray(colb)
        in_maps.append(m)
    return in_maps


def kernel(**inputs):
    from concourse.bass_utils import run_bass_kernel_spmd

    if "nc" not in _CACHE:
        _CACHE["nc"] = _build_program()
    nc = _CACHE["nc"]

    in_maps = _prep_inputs(inputs)
    trace = os.environ.get("KERNEL_TRACE", "0") == "1"
    res = run_bass_kernel_spmd(nc, in_maps, core_ids=list(range(8)),
                               trace=trace)
    if res.exec_time_ns is not None:
        print(f"HW exec time: {res.exec_time_ns} ns")
        _CACHE["exec_time_ns"] = res.exec_time_ns
    out = np.empty((B, S, D), dtype=np.float32)
    for core in range(8):
        b, h = core // 2, core % 2
        out[b, h * WINDOW:(h + 1) * WINDOW] = res.results[core]["y"].T
    return out


# revision 11
# speedup vs baseline: 1.7275x; 1.0516x over previous
"""Trainium2 Bass kernel for a custom transformer block (v2, bf16).

Sharding: 8 cores = 4 batches x 2 sequence halves. Each core computes the
full block (LN1 -> QKV -> windowed attention -> LN2 -> MLP -> residual) for
its 1024 query tokens; the KV window (last 1024 tokens of its batch) is
recomputed on both cores of a batch pair to avoid any collectives.

v2 changes vs the fp32r baseline:
- all matmuls in bf16 (weights host-cast; LN gain/bias folded into the
  weight matrices and effective biases on the host, so device LN is just
  (x-mu)*rstd).
- all 128x128 transposes moved off the PE onto the DMA xbar
  (dma_start_transpose), with contiguous destinations.
- padding mask applied as a per-key bias inside the exp activation
  (exp(s-200)=0 exactly); the causal mask is a bf16 min against
  exp-domain constants applied only on the staircase strip tiles.
- attention probs, q/k/v, z in bf16; psum evacuations on DVE.
- MLP h2 accumulated in PSUM across all 32 contraction chunks (w2 is
  streamed twice, once per query half) with the residual fused into the
  final evacuation.
"""
import sys
import os

if "/opt/trn_rl_repo" not in sys.path:
    sys.path.insert(0, "/opt/trn_rl_repo")

import numpy as np
import ml_dtypes

B, S, D = 4, 2048, 1024
N_HEAD = 16
D_HEAD = 64
WINDOW = 1024
D_FF = 4096
EPS = 1e-5
ISD = float(1.0 / np.sqrt(D))  # 1/32
EMASK = float(np.exp(-80.0))   # exp-domain mask value for causal-masked
EKEEP = 3e38
PADB = -200.0                  # pad-key bias inside exp: exp(s-200) == 0.0
P = 128

_CACHE = {}


def _build_program():
    import concourse.bacc as bacc
    import concourse.mybir as mybir
    from concourse.tile import TileContext

    F32 = mybir.dt.float32
    BF16 = mybir.dt.bfloat16
    AF = mybir.ActivationFunctionType
    ALU = mybir.AluOpType
    AX = mybir.AxisListType

    nc = bacc.Bacc("TRN2", target_bir_lowering=False, debug=False,
                   num_devices=8)

    xin_d = nc.dram_tensor("xin", [2 * WINDOW, D], F32, kind="ExternalInput")
    maskT_d = nc.dram_tensor("maskT", [WINDOW, WINDOW], BF16,
                             kind="ExternalInput")
    wq_d = nc.dram_tensor("wq", [D, D], BF16, kind="ExternalInput")
    wkv_d = nc.dram_tensor("wkv", [D, 2 * D], BF16, kind="ExternalInput")
    w1_d = nc.dram_tensor("w1", [D, D_FF], BF16, kind="ExternalInput")
    w2_d = nc.dram_tensor("w2", [D_FF, D], BF16, kind="ExternalInput")
    bqs_d = nc.dram_tensor("bqs", [P, 8], F32, kind="ExternalInput")
    bkvk_d = nc.dram_tensor("bkvk", [P, 8], F32, kind="ExternalInput")
    bkvvb_d = nc.dram_tensor("bkvvb", [P, D], F32, kind="ExternalInput")
    colb_d = nc.dram_tensor("colb", [P, 8], F32, kind="ExternalInput")
    b1s_d = nc.dram_tensor("b1s", [P, 32], F32, kind="ExternalInput")
    b2s_d = nc.dram_tensor("b2s", [P, 8], F32, kind="ExternalInput")
    xinT_d = nc.dram_tensor("xinT", [D, WINDOW], F32, kind="ExternalInput")
    y_d = nc.dram_tensor("y", [D, WINDOW], F32, kind="ExternalOutput")

    with TileContext(nc) as tc:
        cpool = tc.alloc_tile_pool(name="const", bufs=1, side="left")
        smallc = cpool.tile([P, 64], F32)
        bqs = smallc[:, 0:8]
        bkvk = smallc[:, 8:16]
        b1s = smallc[:, 16:48]
        b2s = smallc[:, 48:56]
        colb = smallc[:, 56:64]
        onesc = cpool.tile([P, 16], F32)
        nc.vector.memset(onesc[:], 1.0)
        nc.sync.dma_start(bqs, bqs_d[:])
        nc.sync.dma_start(bkvk, bkvk_d[:])
        nc.sync.dma_start(b1s, b1s_d[:])
        nc.sync.dma_start(b2s, b2s_d[:])
        nc.sync.dma_start(colb, colb_d[:])
        bkvvb = cpool.tile([P, D], F32)
        nc.sync.dma_start(bkvvb[:], bkvvb_d[:])

        # weights for QKV: issue DMA up front so they land during LN1
        wpool = tc.alloc_tile_pool(name="wqkv", bufs=1, side="left")
        wqr = wpool.tile([P, 8, D], BF16)
        wkvr = wpool.tile([P, 8, 2 * D], BF16)
        for kc in range(8):
            nc.sync.dma_start(wqr[:, kc, :], wq_d[kc * P:(kc + 1) * P, :])
            nc.sync.dma_start(wkvr[:, kc, :], wkv_d[kc * P:(kc + 1) * P, :])

        # ---------------- Phase B: LN1 + transpose to dim-major ------------
        zTp = tc.alloc_tile_pool(name="zT", bufs=1, side="left")
        zqT = zTp.tile([P, 8, 8, P], BF16)   # [dim, tok-tile, dim-chunk, tok]
        zwT = zTp.tile([P, 8, 8, P], BF16)
        xz = tc.alloc_tile_pool(name="xz", bufs=2, side="left")

        def ln1_group(g):
            """Batched LN over tiles 8g..8g+7: per-tile sums feed one batched
            mu/rstd computation (two cross-engine hops for the whole group)."""
            dst = zqT if g == 0 else zwT
            xts = []
            st = xz.tile([P, 6, 8], F32, tag="stats", name=f"st{g}")
            mus, sqs = st[:, 0, :], st[:, 1, :]
            mu, veps = st[:, 2, :], st[:, 3, :]
            sdv, rstd = st[:, 4, :], st[:, 5, :]
            for i in range(8):
                t = g * 8 + i
                xt = xz.tile([P, D], F32, tag=f"x{i}", bufs=1,
                             name=f"x{g}_{i}")
                nc.sync.dma_start(xt[:], xin_d[t * P:(t + 1) * P, :])
                xts.append(xt)
                nc.vector.reduce_sum(mus[:, i:i + 1], xt[:], axis=AX.X)
                sq = xz.tile([P, D], F32, tag="sq", name=f"sq{g}_{i}")
                nc.scalar.activation(sq[:], xt[:], AF.Square,
                                     accum_out=sqs[:, i:i + 1])
            nc.vector.tensor_scalar_mul(mu, mus, 1.0 / D)
            nc.vector.tensor_scalar(veps, sqs, 1.0 / D, EPS,
                                    op0=ALU.mult, op1=ALU.add)
            nc.vector.tensor_tensor(sdv, mu, mu, op=ALU.mult)
            nc.vector.tensor_tensor(veps, veps, sdv, op=ALU.subtract)
            nc.scalar.sqrt(sdv, veps)
            nc.vector.reciprocal(rstd, sdv)
            for i in range(8):
                z = xz.tile([P, D], BF16, tag="z", name=f"z{g}_{i}")
                nc.vector.tensor_scalar(z[:], xts[i][:], mu[:, i:i + 1],
                                        rstd[:, i:i + 1],
                                        op0=ALU.subtract, op1=ALU.mult)
                nc.sync.dma_start_transpose(dst[:, i, :, :], z[:])

        ln1_group(0)
        ln1_group(1)

        qkvp = tc.alloc_tile_pool(name="qkv", bufs=1, side="right")
        qT = qkvp.tile([P, 8, WINDOW], BF16)      # q/sqrt(D), dim-major
        kT = qkvp.tile([P, 8, WINDOW], BF16)      # k, dim-major
        V = qkvp.tile([P, 8, N_HEAD * 65], BF16)  # token-major + ones col

        psC = tc.alloc_tile_pool(name="psC", bufs=8, space="PSUM")

        # Q: weights stationary -> qT dim-major, scaled by 1/32.
        # kc outer so each loaded weight tile serves both query halves.
        for cog in range(2):
            pq = [[psC.tile([P, 512], F32, tag="proj", name=f"pq{cog}_{a}_{b}")
                   for b in range(2)] for a in range(4)]
            for kc in range(8):
                for c4 in range(4):
                    co = cog * 4 + c4
                    for qh in range(2):
                        nc.tensor.matmul(
                            pq[c4][qh][:], wqr[:, kc, co * P:(co + 1) * P],
                            zqT[:, qh * 4:(qh + 1) * 4, kc, :],
                            start=(kc == 0), stop=(kc == 7))
            for c4 in range(4):
                co = cog * 4 + c4
                for qh in range(2):
                    nc.vector.tensor_scalar(
                        qT[:, co, qh * 512:(qh + 1) * 512], pq[c4][qh][:],
                        ISD, bqs[:, co:co + 1], op0=ALU.mult, op1=ALU.add)

        # K: weights stationary -> kT dim-major
        for cog in range(2):
            pk = [[psC.tile([P, 512], F32, tag="proj", name=f"pk{cog}_{a}_{b}")
                   for b in range(2)] for a in range(4)]
            for kc in range(8):
                for c4 in range(4):
                    co = cog * 4 + c4
                    for qh in range(2):
                        nc.tensor.matmul(
                            pk[c4][qh][:], wkvr[:, kc, co * P:(co + 1) * P],
                            zwT[:, qh * 4:(qh + 1) * 4, kc, :],
                            start=(kc == 0), stop=(kc == 7))
            for c4 in range(4):
                co = cog * 4 + c4
                for qh in range(2):
                    nc.vector.tensor_scalar(
                        kT[:, co, qh * 512:(qh + 1) * 512], pk[c4][qh][:],
                        bkvk[:, co:co + 1], None, op0=ALU.add)

        # V: activations stationary -> token-major, bias added at evac
        for ttg in range(2):
            pv = [[psC.tile([P, 512], F32, tag="proj", name=f"pv{ttg}_{a}_{b}")
                   for b in range(2)] for a in range(4)]
            for kc in range(8):
                for t4 in range(4):
                    tt = ttg * 4 + t4
                    for vh in range(2):
                        nc.tensor.matmul(
                            pv[t4][vh][:], zwT[:, tt, kc, :],
                            wkvr[:, kc, D + vh * 512:D + (vh + 1) * 512],
                            start=(kc == 0), stop=(kc == 7))
            for t4 in range(4):
                tt = ttg * 4 + t4
                for vh in range(2):
                    vdst = V[:, tt, :].rearrange("p (h n) -> p h n", n=65)[
                        :, vh * 8:(vh + 1) * 8, 0:64]
                    nc.vector.scalar_tensor_tensor(
                        vdst, pv[t4][vh][:].rearrange("p (h n) -> p h n", n=64),
                        0.0,
                        bkvvb[:, vh * 512:(vh + 1) * 512].rearrange(
                            "p (h n) -> p h n", n=64),
                        op0=ALU.add, op1=ALU.add)
        for tt in range(8):
            nc.scalar.copy(
                V[:, tt, :].rearrange("p (h n) -> p h n", n=65)[:, :, 64:65],
                onesc.rearrange("p (h n) -> p h n", n=1))

        psC.release()
        xz.release()
        zTp.release()
        wpool.release()

        # prefetch first MLP weight chunks during attention
        wf1 = tc.alloc_tile_pool(name="wf1", bufs=2, side="left")
        w1rs = {}
        for sc in range(2):
            w1r = wf1.tile([P, 8, D], BF16, tag="w1r")
            for kc in range(8):
                nc.sync.dma_start(
                    w1r[:, kc, :], w1_d[kc * P:(kc + 1) * P,
                                        sc * 1024:(sc + 1) * 1024])
            w1rs[sc] = w1r

        # ---------------- Phase D: attention --------------------------------
        attnp = tc.alloc_tile_pool(name="attn", bufs=1, side="left")
        attn = attnp.tile([P, 8, D], BF16)         # normalized attn out
        asum = attnp.tile([P, 8, N_HEAD], F32)     # per-head row sums of attn
        rinv = attnp.tile([P, 2, 8], F32)          # per-head 1/rowsum
        oa = attnp.tile([80, 2, WINDOW], BF16)     # AV out + rowsum row
        nc.vector.memset(oa[:, :, :], 0.0)

        mkp = tc.alloc_tile_pool(name="mk", bufs=1, side="left")
        maskT = mkp.tile([P, 8, WINDOW], BF16)
        nc.sync.dma_start(maskT[:], maskT_d.rearrange("(c p) n -> p c n", p=P))
        ptp = tc.alloc_tile_pool(name="ptp", bufs=2, side="left")
        otp = tc.alloc_tile_pool(name="otp", bufs=2, side="left")
        psS = tc.alloc_tile_pool(name="psS", bufs=2, space="PSUM")
        psA = tc.alloc_tile_pool(name="psA", bufs=2, space="PSUM")

        for hp in range(N_HEAD // 2):
            pair = (2 * hp, 2 * hp + 1)
            pts = {0: ptp.tile([P, 8, WINDOW], BF16, tag="pts0", name=f"pts0_{hp}"),
                   1: ptp.tile([P, 8, WINDOW], BF16, tag="pts1", name=f"pts1_{hp}")}
            avp = {0: psA.tile([65, WINDOW], F32, tag="av", name=f"av0_{hp}"),
                   1: psA.tile([65, WINDOW], F32, tag="av", name=f"av1_{hp}")}
            for kc in range(8):
                sps = {0: psS.tile([P, WINDOW], F32, tag="s", name=f"s0_{hp}_{kc}"),
                       1: psS.tile([P, WINDOW], F32, tag="s", name=f"s1_{hp}_{kc}")}
                # alternate PE row halves so LDWEIGHTS overlaps matmul
                for qh in range(2):
                    for i in range(2):
                        po = 64 * i
                        nc.tensor.matmul(
                            sps[i][:, qh * 512:(qh + 1) * 512],
                            kT[po:po + 64, hp, kc * P:(kc + 1) * P],
                            qT[po:po + 64, hp, qh * 512:(qh + 1) * 512],
                            start=True, stop=True)
                for i in range(2):
                    nc.scalar.activation(pts[i][:, kc, :], sps[i][:], AF.Exp,
                                         bias=colb[:, kc:kc + 1])
                w = min(P * (kc + 1), WINDOW)
                for i in range(2):
                    nc.vector.tensor_tensor(
                        pts[i][:, kc, 0:w], pts[i][:, kc, 0:w],
                        maskT[:, kc, 0:w], op=ALU.min)
                for i in range(2):
                    h = pair[i]
                    for qh in range(2):
                        nc.tensor.matmul(
                            avp[i][:, qh * 512:(qh + 1) * 512],
                            V[:, kc, h * 65:(h + 1) * 65],
                            pts[i][:, kc, qh * 512:(qh + 1) * 512],
                            start=(kc == 0), stop=(kc == 7))
            for i in range(2):
                h = pair[i]
                for qh in range(2):
                    nc.vector.tensor_copy(
                        oa[0:65, i, qh * 512:(qh + 1) * 512],
                        avp[i][:, qh * 512:(qh + 1) * 512])
                oT = otp.tile([P, 8, 80], BF16, tag="oT")
                nc.sync.dma_start_transpose(oT[:, :, :], oa[:, i, :])
                nc.vector.reciprocal(rinv[:, i, :], oT[:, :, 64:65])
                for t in range(8):
                    nc.vector.tensor_scalar(
                        attn[:, t, h * 64:(h + 1) * 64], oT[:, t, 0:64],
                        rinv[:, i, t:t + 1], 0.0, op0=ALU.mult,
                        op1=ALU.add, accum_out=asum[:, t, h:h + 1])

        psA.release()
        psS.release()
        otp.release()
        ptp.release()
        mkp.release()
        qkvp.release()

        # ---------------- Phase E: LN2 + transpose ---------------------------
        z2Tp = tc.alloc_tile_pool(name="z2T", bufs=1, side="right")
        z2T = z2Tp.tile([P, 8, 8, P], BF16)
        xz2 = tc.alloc_tile_pool(name="xz2", bufs=2, side="left")
        st2 = xz2.tile([P, 6, 8], F32, tag="stats2")
        mus2, vsum2 = st2[:, 0, :], st2[:, 1, :]
        mu2, veps2 = st2[:, 2, :], st2[:, 3, :]
        sdv2, rstd2 = st2[:, 4, :], st2[:, 5, :]
        for t in range(8):
            nc.vector.reduce_sum(mus2[:, t:t + 1], asum[:, t, :], axis=AX.X)
        nc.vector.tensor_scalar_mul(mu2, mus2, 1.0 / D)
        for t in range(8):
            scr = xz2.tile([P, D], BF16, tag="scrE", name=f"scr{t}")
            nc.vector.scalar_tensor_tensor(
                scr[:], attn[:, t, :], mu2[:, t:t + 1], attn[:, t, :],
                op0=ALU.subtract, op1=ALU.mult, accum_out=vsum2[:, t:t + 1])
        nc.vector.tensor_scalar(veps2, vsum2, 1.0 / D, EPS,
                                op0=ALU.mult, op1=ALU.add)
        nc.scalar.sqrt(sdv2, veps2)
        nc.vector.reciprocal(rstd2, sdv2)
        for t in range(8):
            z2 = xz2.tile([P, D], BF16, tag="z2E", name=f"z2_{t}")
            nc.vector.tensor_scalar(z2[:], attn[:, t, :], mu2[:, t:t + 1],
                                    rstd2[:, t:t + 1],
                                    op0=ALU.subtract, op1=ALU.mult)
            nc.sync.dma_start_transpose(z2T[:, t, :, :], z2[:])

        xz2.release()
        attnp.release()

        # ---------------- Phase F: MLP ---------------------------------------
        h1p = tc.alloc_tile_pool(name="h1p", bufs=1, side="left")
        h1 = h1p.tile([P, 32, WINDOW], BF16)
        psF = tc.alloc_tile_pool(name="psF", bufs=8, space="PSUM")

        for sc in range(4):
            if sc in w1rs:
                w1r = w1rs.pop(sc)
            else:
                w1r = wf1.tile([P, 8, D], BF16, tag="w1r")
                for kc in range(8):
                    nc.sync.dma_start(
                        w1r[:, kc, :], w1_d[kc * P:(kc + 1) * P,
                                            sc * 1024:(sc + 1) * 1024])
            for ftg in range(2):
                ph = [[psF.tile([P, 512], F32, tag="f", name=f"ph{sc}_{ftg}_{a}_{b}")
                       for b in range(2)] for a in range(4)]
                for kc in range(8):
                    for f4 in range(4):
                        ft = ftg * 4 + f4
                        for qh in range(2):
                            nc.tensor.matmul(
                                ph[f4][qh][:],
                                w1r[:, kc, ft * P:(ft + 1) * P],
                                z2T[:, qh * 4:(qh + 1) * 4, kc, :],
                                start=(kc == 0), stop=(kc == 7))
                for f4 in range(4):
                    ft = sc * 8 + ftg * 4 + f4
                    for qh in range(2):
                        nc.scalar.activation(
                            h1[:, ft, qh * 512:(qh + 1) * 512], ph[f4][qh][:],
                            AF.Silu, bias=b1s[:, ft:ft + 1], scale=1.0)

        z2Tp.release()

        # h2: accumulate all 32 contraction chunks in PSUM per (co, qh);
        # w2 is streamed once per query half.
        wf2 = tc.alloc_tile_pool(name="wf2", bufs=2, side="right")
        tailp = tc.alloc_tile_pool(name="tail", bufs=2, side="left")
        xqs = {}
        for qh in range(2):
            for co in range(8):
                xq = tailp.tile([P, 512], F32, tag=f"xq{qh}_{co}", bufs=1,
                                name=f"xq{qh}_{co}")
                nc.sync.dma_start(
                    xq[:], xinT_d[co * P:(co + 1) * P,
                                  qh * 512:(qh + 1) * 512])
                xqs[(qh, co)] = xq
        for qh in range(2):
            ph2 = [psF.tile([P, 512], F32, tag="f", name=f"ph2_{qh}_{a}") for a in range(8)]
            for sc in range(4):
                w2r = wf2.tile([P, 8, D], BF16, tag="w2r")
                for kc in range(8):
                    nc.sync.dma_start(
                        w2r[:, kc, :],
                        w2_d[(sc * 8 + kc) * P:(sc * 8 + kc + 1) * P, :])
                if sc < 3:
                    for kc in range(8):
                        for co in range(8):
                            nc.tensor.matmul(
                                ph2[co][:], w2r[:, kc, co * P:(co + 1) * P],
                                h1[:, sc * 8 + kc, qh * 512:(qh + 1) * 512],
                                start=(sc == 0 and kc == 0), stop=False)
                else:
                    # last chunk co-major so each co finishes (and evacuates)
                    # while the remaining co's still accumulate
                    for co in range(8):
                        for kc in range(8):
                            nc.tensor.matmul(
                                ph2[co][:], w2r[:, kc, co * P:(co + 1) * P],
                                h1[:, sc * 8 + kc, qh * 512:(qh + 1) * 512],
                                start=False, stop=(kc == 7))
                        y = tailp.tile([P, 512], F32, tag="y", name=f"y{qh}_{co}")
                        nc.vector.scalar_tensor_tensor(
                            y[:], ph2[co][:], b2s[:, co:co + 1],
                            xqs[(qh, co)][:], op0=ALU.add, op1=ALU.add)
                        nc.sync.dma_start(
                            y_d[co * P:(co + 1) * P,
                                qh * 512:(qh + 1) * 512], y[:])

        psF.release()
        tailp.release()
        h1p.release()
        wf1.release()
        wf2.release()
        cpool.release()

    nc.compile()
    return nc


def _prep_inputs(inputs):
    x = np.ascontiguousarray(np.asarray(inputs["x"], dtype=np.float32))
    kpm = np.asarray(inputs["key_pad_mask"]).astype(bool)
    wq = np.asarray(inputs["wq"], dtype=np.float32)
    wkv = np.asarray(inputs["wkv"], dtype=np.float32)
    w1 = np.asarray(inputs["w1"], dtype=np.float32)
    w2 = np.asarray(inputs["w2"], dtype=np.float32)
    bq = np.asarray(inputs["bq"], dtype=np.float32)
    bkv = np.asarray(inputs["bkv"], dtype=np.float32)
    b1 = np.asarray(inputs["b1"], dtype=np.float32)
    b2 = np.asarray(inputs["b2"], dtype=np.float32)
    ln1_g = np.asarray(inputs["ln1_g"], dtype=np.float32)
    ln1_b = np.asarray(inputs["ln1_b"], dtype=np.float32)
    ln2_g = np.asarray(inputs["ln2_g"], dtype=np.float32)
    ln2_b = np.asarray(inputs["ln2_b"], dtype=np.float32)

    # fold LN gains into the weights and LN biases into effective biases
    wq_f = wq * ln1_g[:, None]
    wkv_f = wkv * ln1_g[:, None]
    w1_f = w1 * ln2_g[:, None]
    bq_eff = ln1_b @ wq + bq
    bkv_eff = ln1_b @ wkv + bkv
    b1_eff = ln2_b @ w1 + b1

    def bf(a):
        return np.ascontiguousarray(a.astype(ml_dtypes.bfloat16))

    def dm(v):  # [D] -> [P, 8] dim-major chunk layout
        return np.ascontiguousarray(v.reshape(8, P).T)

    shared = {
        "wq": bf(wq_f),
        "wkv": bf(wkv_f),
        "w1": bf(w1_f),
        "w2": bf(w2),
        "bqs": np.ascontiguousarray((bq_eff * ISD).reshape(8, P).T),
        "bkvk": dm(bkv_eff[0:D]),
        "bkvvb": np.ascontiguousarray(
            np.broadcast_to(bkv_eff[D:2 * D], (P, D)).astype(np.float32)),
        "b1s": np.ascontiguousarray(b1_eff.reshape(32, P).T),
        "b2s": dm(b2),
    }

    j = np.arange(WINDOW)[:, None]   # key index within window (row)
    i = np.arange(WINDOW)[None, :]   # local query index (col)
    in_maps = []
    for core in range(8):
        b, h = core // 2, core % 2
        xq = x[b, h * WINDOW:(h + 1) * WINDOW]
        xw = x[b, S - WINDOW:S]
        causal = j > h * WINDOW + i
        maskT = np.where(causal, np.float32(EMASK),
                         np.float32(EKEEP)).astype(ml_dtypes.bfloat16)
        colb = np.where(kpm[b, S - WINDOW:S], np.float32(PADB),
                        np.float32(0.0)).reshape(8, P).T
        m = dict(shared)
        m["xin"] = np.ascontiguousarray(np.concatenate([xq, xw], axis=0))
        m["xinT"] = np.ascontiguousarray(xq.T)
        m["maskT"] = np.ascontiguousarray(maskT)
        m["colb"] = np.ascontiguousarray(colb)
        in_maps.append(m)
    return in_maps


def kernel(**inputs):
    from concourse.bass_utils import run_bass_kernel_spmd

    if "nc" not in _CACHE:
        _CACHE["nc"] = _build_program()
    nc = _CACHE["nc"]

    in_maps = _prep_inputs(inputs)
    trace = os.environ.get("KERNEL_TRACE", "0") == "1"
    res = run_bass_kernel_spmd(nc, in_maps, core_ids=list(range(8)),
                               trace=trace)
    if res.exec_time_ns is not None:
        print(f"HW exec time: {res.exec_time_ns} ns")
        _CACHE["exec_time_ns"] = res.exec_time_ns
    out = np.empty((B, S, D), dtype=np.float32)
    for core in range(8):
        b, h = core // 2, core % 2
        out[b, h * WINDOW:(h + 1) * WINDOW] = res.results[core]["y"].T
    return out
